# revision 43
# baseline (speedup 1.0000x reference)
"""CQAttention Trainium2 Bass kernel.

Computes, per batch b (B=128, D=128, LC=400, LQ=50):
    S = Wc.C (over rows) + Wq.Q (over cols) + Wqc.(C*Q)   [LC, LQ]
    S1 = softmax(S, axis=LQ); S2 = softmax(S, axis=LC)
    A  = Q @ S1^T                    [D, LC]
    Bm = (C @ S2) @ S1^T             [D, LC]
    out = concat([C, A, C*A, C*Bm])  [4D, LC]

Sharding: data-parallel over batch, 16 batches per core x 8 cores.

Kernel math (per batch, ST = S^T layout [LQ=50 part, LC=400 free]):
    QW[d,j]  = Wqc[d]*Q[d,j] + Wc[d]                (DVE 2-scalar)
    ST'      = QW^T @ C                 [50,400]    (folds the Wc.C row term)
    cT[j]    = Q^T @ Wq                 [50,1]
    expST    = exp(ST' + cT)  (+row sums den2)      (ACT, accum_out)
    d1b      = ones[50,50]^T @ expST    [50,400]    (bcast column sums over j)
    S1T      = expST / d1b                          (softmax over j, unnormed-i)
    expS/CT  = PE transposes of expST / C, 4 full 128-chunks (zero-padded
               to 512 cols so every chunk is complete)
    T1T_raw  = sum_c expS_c^T @ CT_c    [50,128]
    T1T      = T1T_raw * recip(den2)                (softmax over i, folded)
    A        = QT^T @ S1T               [128,400]
    Bm       = T1T^T @ S1T              [128,400]
    out rows: [C, A, C*A, C*Bm]
"""

import os
import sys
import time

# The kernel executes via the axon PJRT backend; make sure it isn't masked
# by an explicit cpu pin (harmless if jax is already initialized with axon).
_jp = os.environ.get("JAX_PLATFORMS", "")
if _jp and "axon" not in _jp:
    os.environ["JAX_PLATFORMS"] = "axon," + _jp

for _p in ("/opt/trn_rl_repo", "/root/.axon_site/_ro/trn_rl_repo"):
    if _p not in sys.path:
        sys.path.append(_p)

import numpy as np

B, D, LC, LQ = 128, 128, 400, 50
N_CORES = 8
BPC = B // N_CORES  # 16 batches per core
LCP = 512           # padded LC (4 full 128-wide transpose chunks)

# fp32r runs the big N>=256 matmuls at full PE rate (4x over fp32) with
# reduced mantissa on HW. Flip for exact fp32.
USE_F32R_BIG = True
USE_F32R_T1T = False  # free: N=128 matmuls cost the same in fp32
# s1t via a single tensor_tensor divide vs reciprocal_approx_fast + mul.
# (divide is rejected by walrus codegen on TRN2 — keep False)
USE_TT_DIVIDE = False


def build_nc(bpc=BPC, use_f32r_big=USE_F32R_BIG, use_f32r_t1t=USE_F32R_T1T,
             use_tt_divide=USE_TT_DIVIDE, enable_asserts=False,
             mid_bufs=7, outp_bufs=6, io_bufs=5, qw_pool_engine=True,
             c_halves=5, e_slots=5, pb=(2, 1, 2, 1, 2, 0),
             t1t_bf16=False, o1_on_act=False, t1t_scale_on_dve=False,
             ct_on_act=False, s1t_on_pool=False, split_store=False,
             load_group=2, o2_on_pool=False, o3_via_act_pool=False,
             o1_split=False, fuse_o23=False,
             pool_alloc_mode="stack", detect_races=True):
    import concourse.bacc as bacc
    import concourse.tile as tile
    from concourse import mybir
    from concourse.masks import make_identity

    F32 = mybir.dt.float32
    F32R = mybir.dt.float32r
    AFT = mybir.ActivationFunctionType
    ALU = mybir.AluOpType

    FR_BIG = F32R if use_f32r_big else F32
    FR_T1T = mybir.dt.bfloat16 if t1t_bf16 else (
        F32R if use_f32r_t1t else F32)

    def f32v(ap):  # plain-f32 view (for transposes of f32r tiles)
        return ap.bitcast(F32) if ap.dtype == F32R else ap

    assert bpc % 2 == 0
    # load_group > 2 would overrun cbuf at the half-index wraparound
    assert load_group == 2
    nc = bacc.Bacc("TRN2", target_bir_lowering=False, debug=False,
                   enable_asserts=enable_asserts, num_devices=N_CORES,
                   detect_race_conditions=detect_races)
    C_ap = nc.dram_tensor("C", [bpc, D, LC], F32, kind="ExternalInput").ap()
    Q_ap = nc.dram_tensor("Q", [bpc, D, LQ], F32, kind="ExternalInput").ap()
    W_ap = nc.dram_tensor("W", [bpc, 1, 3 * D], F32, kind="ExternalInput").ap()
    out_ap = nc.dram_tensor("out", [bpc, 4 * D, LC], F32,
                            kind="ExternalOutput").ap()

    with tile.TileContext(nc, pool_alloc_mode=pool_alloc_mode) as tc:
        from contextlib import ExitStack
        with ExitStack() as ctx:
            consts = ctx.enter_context(tc.tile_pool(name="consts", bufs=1))
            io = ctx.enter_context(tc.tile_pool(name="io", bufs=io_bufs))
            mid = ctx.enter_context(tc.tile_pool(name="mid", bufs=mid_bufs))
            outp = ctx.enter_context(tc.tile_pool(name="outp", bufs=outp_bufs))
            pp_st = ctx.enter_context(
                tc.tile_pool(name="pp_st", bufs=pb[0], space="PSUM"))
            pp_small = (ctx.enter_context(
                tc.tile_pool(name="pp_small", bufs=pb[1], space="PSUM"))
                if pb[1] else None)
            pp_tr = ctx.enter_context(
                tc.tile_pool(name="pp_tr", bufs=pb[2], space="PSUM"))
            pp_t1t = ctx.enter_context(
                tc.tile_pool(name="pp_t1t", bufs=pb[3], space="PSUM"))
            pp_ab = ctx.enter_context(
                tc.tile_pool(name="pp_ab", bufs=pb[4], space="PSUM"))
            pp_es = (ctx.enter_context(
                tc.tile_pool(name="pp_es", bufs=pb[5], space="PSUM"))
                if len(pb) > 5 and pb[5] else None)

            # --- constants ---
            ident = consts.tile([128, 128], F32)
            make_identity(nc, ident)
            ones_f32 = consts.tile([LQ, LQ], F32)
            nc.vector.memset(ones_f32, 1.0)
            onesmat = consts.tile([LQ, LQ], FR_BIG)
            nc.vector.tensor_copy(onesmat, ones_f32)

            # --- W preload: [bpc,384] -> per-d columns [128, 3*bpc] ---
            w_stage = consts.tile([bpc, 3 * D], F32)
            nc.sync.dma_start(w_stage, W_ap[:, 0, :])
            wTp = pp_tr.tile([128, 3 * bpc], F32, tag="tr")
            for k in range(3):
                nc.tensor.matmul(
                    wTp[:, k * bpc:(k + 1) * bpc],
                    w_stage[:, k * D:(k + 1) * D],
                    ident[:bpc, :bpc],
                    is_transpose=True, start=True, stop=True)
            w_all = consts.tile([128, 3 * bpc], F32)
            nc.vector.tensor_copy(w_all, wTp)

            # Manually double-buffered C-pair and expST tiles: persistent
            # allocations so the pad columns [LC:LCP] can be zeroed exactly
            # once. Loads/exp only ever write [:, :LC]; chunk-3 transposes
            # then read defined zeros from the pads. Tile still tracks
            # per-region deps on the halves, so pipelining is preserved.
            cbuf = consts.tile([D, c_halves * 2 * LCP], F32)
            nc.gpsimd.memset(
                cbuf[:].rearrange("p (t s) -> p t s",
                                  t=2 * c_halves)[:, :, LC:], 0.0)
            ebuf_f32 = consts.tile([LQ, e_slots * LCP], F32)
            nc.gpsimd.memset(
                ebuf_f32[:].rearrange("p (t s) -> p t s",
                                      t=e_slots)[:, :, LC:],
                0.0)
            ebuf = ebuf_f32[:].bitcast(FR_BIG) if FR_BIG != F32 else ebuf_f32[:]

            qpair = None
            for b in range(bpc):
                wq_col = w_all[:, b:b + 1]
                wc_col = w_all[:, bpc + b:bpc + b + 1]
                wqc_col = w_all[:, 2 * bpc + b:2 * bpc + b + 1]

                k = b % 2
                half = (b // 2) % c_halves
                cpair = cbuf[:, half * 2 * LCP:(half + 1) * 2 * LCP]
                if b % load_group == 0:
                    # grouped loads: load_group batches per DMA, written
                    # across the consecutive pair-halves they will occupy
                    g = load_group
                    h0 = (b // 2) % c_halves
                    dst = cbuf[:, h0 * 2 * LCP:h0 * 2 * LCP + g * LCP]
                    nc.sync.dma_start(
                        dst.rearrange("p (t s) -> p t s", t=g)[:, :, :LC],
                        C_ap[b:b + g].rearrange("t d i -> d t i"))
                    qpair = io.tile([D, g * LQ], F32, tag="qpair")
                    nc.sync.dma_start(
                        qpair[:].rearrange("p (t s) -> p t s", t=g),
                        Q_ap[b:b + g].rearrange("t d j -> d t j"))
                ct = cpair[:, k * LCP:(k + 1) * LCP]     # [128, 512] padded
                qt = qpair[:, (b % load_group) * LQ:
                           (b % load_group + 1) * LQ]    # [128, 50]

                # QW = Wqc*Q + Wc  (DVE two-scalar)
                qw = mid.tile([D, LQ], F32, tag="qw")
                qw_eng = nc.gpsimd if qw_pool_engine else nc.vector
                qw_eng.tensor_scalar(qw, qt, wqc_col, wc_col,
                                     ALU.mult, ALU.add)

                # ST' = QW^T @ C [50,400]; cT = Q^T @ Wq packed into the
                # spare bytes of the same PSUM bank (1600B used of 2048B)
                stp_full = pp_st.tile([LQ, LC + 4], F32, tag="st",
                                      name="stp")
                stp = stp_full[:, :LC]
                nc.tensor.matmul(stp, qw, ct[:, :LC],
                                 start=True, stop=True)
                ctp = (pp_small.tile([LQ, 1], F32, tag="small", name="ctp")
                       if pp_small is not None else stp_full[:, LC:LC + 1])
                nc.tensor.matmul(ctp, qt, wq_col, start=True, stop=True)
                ct_sb = mid.tile([LQ, 1], F32, tag="ctsb")
                (nc.scalar.copy if ct_on_act else nc.vector.tensor_copy)(
                    ct_sb, ctp)

                # expST = exp(ST' + cT), den2 = row sums; padded to 512 cols
                eslot = b % e_slots
                expst = ebuf[:, eslot * LCP:(eslot + 1) * LCP]
                den2 = mid.tile([LQ, 1], F32, tag="den2")
                nc.scalar.activation(expst[:, :LC], stp, AFT.Exp, bias=ct_sb,
                                     accum_out=den2)
                r2 = mid.tile([LQ, 1], F32, tag="r2")
                nc.vector.reciprocal_approx_fast(r2, den2)

                # d1b[j,i] = sum_j' expST[j',i]  (bcast over j)
                d1b = pp_st.tile([LQ, LC + 4], F32, tag="st",
                                 name="d1b")[:, :LC]
                nc.tensor.matmul(d1b, onesmat, expst[:, :LC],
                                 start=True, stop=True)

                # S1T = expST / d1b
                s1t = mid.tile([LQ, LC], FR_BIG, tag="s1t")
                if use_tt_divide:
                    nc.vector.tensor_tensor(s1t, f32v(expst[:, :LC]), d1b,
                                            ALU.divide)
                else:
                    r1b = mid.tile([LQ, LC], F32, tag="r1b")
                    nc.vector.reciprocal_approx_fast(r1b, d1b)
                    s1t_eng = nc.gpsimd if s1t_on_pool else nc.vector
                    s1t_eng.tensor_mul(s1t, f32v(expst[:, :LC]), r1b)

                # QT transpose [50,128]
                qtp = pp_tr.tile([LQ, D], F32, tag="tr")
                nc.tensor.matmul(qtp, qt, ident, is_transpose=True,
                                 start=True, stop=True)
                qt_sb = mid.tile([LQ, D], FR_BIG, tag="qtsb")
                nc.scalar.copy(qt_sb, qtp)

                # C^T chunks into one bank [128, 512] (4 full chunks)
                ctTp = pp_tr.tile([128, 512], F32, tag="tr")
                for c in range(4):
                    nc.tensor.matmul(ctTp[:, c * 128:(c + 1) * 128],
                                     ct[:, c * 128:(c + 1) * 128], ident,
                                     is_transpose=True,
                                     start=True, stop=True)
                ctT_sb = mid.tile([128, 512], FR_T1T, tag="ctTsb")
                nc.scalar.copy(ctT_sb, ctTp)

                # expS chunks into one bank [128, 200]
                esp = ((pp_es or pp_tr).tile(
                    [128, 4 * LQ], F32,
                    tag="es" if pp_es else "tr"))
                for c in range(4):
                    nc.tensor.matmul(esp[:, c * LQ:(c + 1) * LQ],
                                     f32v(expst[:, c * 128:(c + 1) * 128]),
                                     ident[:LQ, :LQ],
                                     is_transpose=True,
                                     start=True, stop=True)
                es_sb = mid.tile([128, 4 * LQ], FR_T1T, tag="essb")
                nc.scalar.copy(es_sb, esp)

                # T1T_raw = sum_c expS_c^T @ CT_c  [50,128]
                t1tp = pp_t1t.tile([LQ, D], F32, tag="t1t")
                for c in range(4):
                    nc.tensor.matmul(
                        t1tp,
                        es_sb[:, c * LQ:(c + 1) * LQ],
                        ctT_sb[:, c * 128:(c + 1) * 128],
                        start=(c == 0), stop=(c == 3))
                t1t_sb = mid.tile([LQ, D], FR_BIG, tag="t1tsb")
                if t1t_scale_on_dve:
                    nc.vector.tensor_scalar(t1t_sb, t1tp, r2, None, ALU.mult)
                else:
                    nc.scalar.mul(t1t_sb, t1tp, r2)

                # A = QT^T @ S1T ; Bm = T1T^T @ S1T  [128,400]
                if fuse_o23:
                    # one double-wide PSUM tile: A in bank0, Bm in bank1
                    abt = pp_ab.tile([D, 1024], F32, tag="ab")
                    a_ps = abt[:, 0:LC]
                    bm_ps = abt[:, 512:512 + LC]
                else:
                    a_ps = pp_ab.tile([D, LC], F32, tag="ab")
                    bm_ps = pp_ab.tile([D, LC], F32, tag="ab")
                nc.tensor.matmul(a_ps, qt_sb, s1t,
                                 start=True, stop=True)
                nc.tensor.matmul(bm_ps, t1t_sb, s1t,
                                 start=True, stop=True)

                # outputs: o1|o2|o3 packed for a single merged store
                outbuf = outp.tile([D, 3 * LC], F32, tag="o")
                if o1_split:
                    nc.vector.tensor_copy(outbuf[:, :LC // 2],
                                          a_ps[:, :LC // 2])
                    nc.scalar.copy(outbuf[:, LC // 2:LC], a_ps[:, LC // 2:])
                else:
                    (nc.scalar.copy if o1_on_act else nc.vector.tensor_copy)(
                        outbuf[:, :LC], a_ps)
                if fuse_o23:
                    # single DVE pass: [C*A | C*Bm] via 3D APs; ct free-dim
                    # broadcast over the two PSUM banks
                    nc.vector.tensor_mul(
                        outbuf[:, LC:].rearrange("p (t s) -> p t s", t=2),
                        ct[:, :LC].unsqueeze(1).broadcast_to([D, 2, LC]),
                        abt[:].rearrange("p (t s) -> p t s", t=2)[:, :, :LC])
                elif o2_on_pool:
                    # o1 is already A in SBUF; Pool can do the SBUF-only mul
                    nc.gpsimd.tensor_mul(outbuf[:, LC:2 * LC], ct[:, :LC],
                                         outbuf[:, :LC])
                    nc.vector.tensor_mul(outbuf[:, 2 * LC:], ct[:, :LC],
                                         bm_ps)
                else:
                    nc.vector.tensor_mul(outbuf[:, LC:2 * LC], ct[:, :LC],
                                         a_ps)
                    if o3_via_act_pool:
                        bm_sb = mid.tile([D, LC], F32, tag="bmsb")
                        nc.scalar.copy(bm_sb, bm_ps)
                        nc.gpsimd.tensor_mul(outbuf[:, 2 * LC:], ct[:, :LC],
                                             bm_sb)
                    else:
                        nc.vector.tensor_mul(outbuf[:, 2 * LC:], ct[:, :LC],
                                             bm_ps)

                nc.sync.dma_start(out_ap[b, 0:D, :], ct[:, :LC])
                if split_store:
                    nc.sync.dma_start(
                        out_ap[b, D:3 * D, :].rearrange(
                            "(t d) i -> d t i", t=2),
                        outbuf[:, :2 * LC].rearrange(
                            "p (t s) -> p t s", t=2))
                    nc.sync.dma_start(out_ap[b, 3 * D:, :],
                                      outbuf[:, 2 * LC:])
                else:
                    nc.sync.dma_start(
                        out_ap[b, D:, :].rearrange("(t d) i -> d t i", t=3),
                        outbuf[:].rearrange("p (t s) -> p t s", t=3))

    nc.compile()
    return nc


_NC_CACHE = {}
last_exec_s = None


def _get_nc():
    key = (BPC, USE_F32R_BIG, USE_F32R_T1T, USE_TT_DIVIDE)
    if key not in _NC_CACHE:
        _NC_CACHE[key] = build_nc()
    return _NC_CACHE[key]


_EXEC_CACHE = {}


def _get_exec():
    """Build (once) a cached sharded PJRT callable for the kernel NEFF.

    Mirrors concourse.bass2jax.run_bass_via_pjrt's multi-core path, but
    caches the jitted function across calls and creates the donated
    output zero-buffers on-device (no 100MB host->device transfer of
    zeros per invocation).
    """
    if "fn" in _EXEC_CACHE:
        return _EXEC_CACHE
    import jax
    from jax.sharding import Mesh, PartitionSpec
    from jax.experimental.shard_map import shard_map
    from concourse import bass2jax, mybir
    from concourse.bass2jax import _bass_exec_p, partition_id_tensor

    bass2jax.install_neuronx_cc_hook()
    nc = _get_nc()

    partition_name = (nc.partition_id_tensor.name
                      if nc.partition_id_tensor else None)
    in_names, out_names, out_avals = [], [], []
    for alloc in nc.m.functions[0].allocations:
        if not isinstance(alloc, mybir.MemoryLocationSet):
            continue
        name = alloc.memorylocations[0].name
        if alloc.kind == "ExternalInput":
            if name != partition_name:
                in_names.append(name)
        elif alloc.kind == "ExternalOutput":
            out_names.append(name)
            out_avals.append(jax.core.ShapedArray(
                tuple(alloc.tensor_shape), mybir.dt.np(alloc.dtype)))
    n_params = len(in_names)
    all_in_names = list(in_names) + list(out_names)
    if partition_name is not None:
        all_in_names.append(partition_name)

    def _body(*args):
        operands = list(args)
        if partition_name is not None:
            operands.append(partition_id_tensor())
        outs = _bass_exec_p.bind(
            *operands,
            out_avals=tuple(out_avals),
            in_names=tuple(all_in_names),
            out_names=tuple(out_names),
            lowering_input_output_aliases=(),
            sim_require_finite=True,
            sim_require_nnan=True,
            nc=nc,
        )
        return tuple(outs)

    try:
        devices = jax.devices("axon")[:N_CORES]
    except Exception:
        devices = jax.devices()[:N_CORES]
    assert len(devices) >= N_CORES, f"need {N_CORES} cores, got {devices}"
    mesh = Mesh(np.asarray(devices[:N_CORES]), ("core",))
    n_outs = len(out_avals)
    donate = tuple(range(n_params, n_params + n_outs))
    in_specs = (PartitionSpec("core"),) * (n_params + n_outs)
    out_specs = (PartitionSpec("core"),) * n_outs
    fn = jax.jit(
        shard_map(_body, mesh=mesh, in_specs=in_specs, out_specs=out_specs,
                  check_rep=False),
        donate_argnums=donate, keep_unused=True)

    from jax.sharding import NamedSharding
    zero_shardings = [NamedSharding(mesh, PartitionSpec("core"))] * n_outs
    zero_shapes = [(N_CORES * a.shape[0], *a.shape[1:]) for a in out_avals]
    zero_dtypes = [a.dtype for a in out_avals]

    import jax.numpy as jnp
    make_zeros = jax.jit(
        lambda: tuple(jnp.zeros(s, d) for s, d in
                      zip(zero_shapes, zero_dtypes)),
        out_shardings=tuple(zero_shardings))

    _EXEC_CACHE.update(dict(fn=fn, in_names=in_names, out_names=out_names,
                            out_avals=out_avals, make_zeros=make_zeros,
                            mesh=mesh))
    return _EXEC_CACHE


def kernel(C, Q, W):
    global last_exec_s
    C = np.ascontiguousarray(C, dtype=np.float32)
    Q = np.ascontiguousarray(Q, dtype=np.float32)
    W = np.ascontiguousarray(W, dtype=np.float32)
    assert C.shape == (B, D, LC) and Q.shape == (B, D, LQ)
    assert W.shape == (B, 1, 3 * D)

    ex = _get_exec()
    full = {"C": C, "Q": Q, "W": W}
    ins = [full[n] for n in ex["in_names"]]
    t0 = time.monotonic()
    zeros = ex["make_zeros"]()
    out_arrs = ex["fn"](*ins, *zeros)
    out_arrs = [np.asarray(o) for o in out_arrs]
    last_exec_s = time.monotonic() - t0
    (oidx,) = [i for i, n in enumerate(ex["out_names"]) if n == "out"]
    return out_arrs[oidx].reshape(B, 4 * D, LC)
